# revision 11
# baseline (speedup 1.0000x reference)
"""Bahdanau attention Trainium2 kernel.

Problem: B=32, S=2048, H=1024 (fp32)
  q_proj = query @ Wa_w.T + Wa_b                  [B, H]
  k_proj = keys @ Ua_w.T + Ua_b                   [B, S, H]
  scores = tanh(q_proj + k_proj) @ Va_w[0] + Va_b [B, S]
  weights = softmax(scores, axis=1)               [B, 1, S]
  context = weights @ keys                        [B, 1, H]
returns (context, weights)

Sharding: data-parallel over batch, 4 examples per core on 8 cores.

Per-core device strategy (all matmuls in bf16, fp32 accumulation):
  - keys DMA'd twice per example: natural layout [s, h] (for context) and
    transposed [h, s] via the xbar DMA-transpose (for k_proj, which
    contracts over h so h must be on partitions).
  - k_proj computed per (o-block 128, s-chunk 512) into PSUM; ScalarE does
    tanh fused with the per-partition bias (q_proj[o] + Wa_b[o] + Ua_b[o])
    writing bf16 to SBUF.
  - scores via PE: Va as a [128,1] stationary column, contract o-blocks.
  - softmax on a single partition row [1, 2048] (Va_b omitted: softmax is
    shift-invariant so it cannot affect either output).
  - column-layout rearrangements (q_proj row -> per-partition bias columns,
    weights row -> [128, 16] stationary column) go through small DRAM
    round-trip DMAs: PE-transpose would put >1 sem wait on the S3_LW
    struct, which walrus rejects.
"""

import os
import sys

sys.path.insert(0, "/opt/trn_rl_repo")

import numpy as np
import ml_dtypes

B, S, H = 32, 2048, 1024
NCORES = 8
BPC = B // NCORES  # examples per core = 4
HB = H // 128      # h (and o) 128-blocks = 8
SBLK = S // 128    # s 128-blocks = 16
SC = 4             # s-chunks per example
SCW = S // SC      # s-chunk width = 512

_BF16 = ml_dtypes.bfloat16

_CACHE = {}
LAST_RESULTS = None  # test harness reads profile/exec time from here


def _build():
    import concourse.bacc as bacc
    import concourse.tile as tile
    from concourse import mybir

    f32 = mybir.dt.float32
    bf16 = mybir.dt.bfloat16
    AF = mybir.ActivationFunctionType
    AX = mybir.AxisListType

    nc = bacc.Bacc("TRN2", target_bir_lowering=False, debug=False)

    keysb = nc.dram_tensor("keysb", [BPC, S, H], bf16, kind="ExternalInput").ap()
    # waT [h, o] and queryT [h, b] packed along the free dim -> one DMA,
    # one semaphore for the q_proj matmul group.
    wq = nc.dram_tensor("wq", [H, H + BPC], bf16, kind="ExternalInput").ap()
    uaT = nc.dram_tensor("uaT", [H, H], bf16, kind="ExternalInput").ap()
    vacol = nc.dram_tensor("vacol", [128, HB], bf16, kind="ExternalInput").ap()
    biasc = nc.dram_tensor("biasc", [128, HB], f32, kind="ExternalInput").ap()
    out_ctx = nc.dram_tensor("out_ctx", [BPC, H], f32, kind="ExternalOutput").ap()
    out_w = nc.dram_tensor("out_w", [BPC, S], f32, kind="ExternalOutput").ap()
    # DRAM scratch for row->column rearrangement round trips
    qp_dram = nc.dram_tensor("qp_dram", [BPC, H], f32, kind="Internal").ap()
    w_dram = nc.dram_tensor("w_dram", [BPC, S], f32, kind="Internal").ap()

    with tile.TileContext(nc) as tc:
        with (
            tc.tile_pool(name="consts", bufs=1) as consts,
            tc.tile_pool(name="keys", bufs=2) as keys_pool,
            tc.tile_pool(name="th", bufs=2) as th_pool,
            tc.tile_pool(name="sm", bufs=2) as sm_pool,
            tc.tile_pool(name="pkp", bufs=2, space="PSUM") as pkp_pool,
            tc.tile_pool(name="psc", bufs=2, space="PSUM") as psc_pool,
            tc.tile_pool(name="pacc", bufs=2, space="PSUM") as pacc_pool,
        ):
            # ---- constants ----
            uaT_sb = consts.tile([128, HB, H], bf16)
            nc.sync.dma_start(out=uaT_sb, in_=uaT.rearrange("(i p) o -> p i o", p=128))
            wq_sb = consts.tile([128, HB, H + BPC], bf16)
            nc.sync.dma_start(out=wq_sb, in_=wq.rearrange("(i p) o -> p i o", p=128))
            vacol_sb = consts.tile([128, HB], bf16)
            nc.sync.dma_start(out=vacol_sb, in_=vacol)
            biasc_sb = consts.tile([128, HB], f32)
            nc.sync.dma_start(out=biasc_sb, in_=biasc)

            # ---- q_proj for all examples: psum_qp[b, o] = queryT.T @ waT ----
            psum_qp = pacc_pool.tile([BPC, H], f32, tag="acc")
            for n in range(2):
                for i in range(HB):
                    nc.tensor.matmul(
                        psum_qp[:, n * 512 : (n + 1) * 512],
                        lhsT=wq_sb[:, i, H : H + BPC],
                        rhs=wq_sb[:, i, n * 512 : (n + 1) * 512],
                        start=(i == 0),
                        stop=(i == HB - 1),
                    )
            qp_sb = sm_pool.tile([BPC, H], f32, tag="qp", bufs=1)
            nc.vector.tensor_copy(qp_sb, psum_qp)
            # row -> column layout via DRAM round trip, then add combined bias
            nc.sync.dma_start(out=qp_dram, in_=qp_sb)
            qpT_raw = consts.tile([128, BPC, HB], f32)
            with nc.allow_non_contiguous_dma("one-time 16KB gather"):
                nc.sync.dma_start(
                    out=qpT_raw, in_=qp_dram.rearrange("b (j p) -> p b j", p=128)
                )
            qpT_sb = consts.tile([128, BPC, HB], f32)
            for j in range(HB):
                nc.vector.tensor_scalar_add(
                    qpT_sb[:, :, j], qpT_raw[:, :, j], biasc_sb[:, j : j + 1]
                )

            # ---- per-example main loop ----
            for b in range(BPC):
                knat = keys_pool.tile([128, SBLK, H], bf16, tag="knat", bufs=2)
                nc.sync.dma_start(
                    out=knat, in_=keysb[b].rearrange("(k p) h -> p k h", p=128)
                )

                scores = sm_pool.tile([1, S], f32, tag="scores", bufs=1)
                for sc in range(SC):
                    # one tile per h-block so each matmul waits on exactly one
                    # transpose-DMA completion
                    kTs = []
                    for i in range(HB):
                        kTi = keys_pool.tile(
                            [128, SCW], bf16, tag=f"kT{i}", bufs=2, name=f"kT{i}"
                        )
                        nc.sync.dma_start_transpose(
                            out=kTi,
                            in_=keysb[
                                b, sc * SCW : (sc + 1) * SCW, i * 128 : (i + 1) * 128
                            ],
                        )
                        kTs.append(kTi)
                    th = th_pool.tile([128, HB, SCW], bf16, tag="th")
                    for j in range(HB):
                        pk = pkp_pool.tile([128, SCW], f32, tag="pk")
                        for i in range(HB):
                            nc.tensor.matmul(
                                pk,
                                lhsT=uaT_sb[:, i, j * 128 : (j + 1) * 128],
                                rhs=kTs[i],
                                start=(i == 0),
                                stop=(i == HB - 1),
                            )
                        nc.scalar.activation(
                            th[:, j, :], pk, AF.Tanh, bias=qpT_sb[:, b, j : j + 1]
                        )
                    ps = psc_pool.tile([1, SCW], f32, tag="ps")
                    for j in range(HB):
                        nc.tensor.matmul(
                            ps,
                            lhsT=vacol_sb[:, j : j + 1],
                            rhs=th[:, j, :],
                            start=(j == 0),
                            stop=(j == HB - 1),
                        )
                    nc.vector.tensor_copy(scores[:, sc * SCW : (sc + 1) * SCW], ps)

                # softmax on [1, S] (single partition)
                nmax = sm_pool.tile([1, 1], f32, tag="nmax")
                nc.vector.reduce_max(nmax, scores, axis=AX.X, negate=True)
                e = sm_pool.tile([1, S], f32, tag="e", bufs=1)
                esum = sm_pool.tile([1, 1], f32, tag="esum")
                nc.scalar.activation(e, scores, AF.Exp, bias=nmax, accum_out=esum)
                rsum = sm_pool.tile([1, 1], f32, tag="rsum")
                nc.vector.reciprocal(rsum, esum)
                wts = sm_pool.tile([1, S], f32, tag="wts", bufs=1)
                nc.vector.tensor_scalar_mul(wts, e, rsum)
                nc.sync.dma_start(out=out_w[b : b + 1], in_=wts)

                # weights row -> bf16 column tile [128, SBLK] via DRAM round trip
                nc.sync.dma_start(out=w_dram[b : b + 1], in_=wts)
                wcol_f = sm_pool.tile([128, SBLK], f32, tag="wcol_f")
                with nc.allow_non_contiguous_dma("8KB gather per example"):
                    nc.sync.dma_start(
                        out=wcol_f, in_=w_dram[b].rearrange("(k p) -> p k", p=128)
                    )
                wcol = sm_pool.tile([128, SBLK], bf16, tag="wcol")
                nc.vector.tensor_copy(wcol, wcol_f)

                # context = sum_s w[s] keys[s, :]
                pctx = pacc_pool.tile([1, H], f32, tag="acc")
                for n in range(2):
                    for k in range(SBLK):
                        nc.tensor.matmul(
                            pctx[:, n * 512 : (n + 1) * 512],
                            lhsT=wcol[:, k : k + 1],
                            rhs=knat[:, k, n * 512 : (n + 1) * 512],
                            start=(k == 0),
                            stop=(k == SBLK - 1),
                        )
                ctx_sb = sm_pool.tile([1, H], f32, tag="ctx", bufs=1)
                nc.vector.tensor_copy(ctx_sb, pctx)
                nc.sync.dma_start(out=out_ctx[b : b + 1], in_=ctx_sb)

    nc.compile()
    return nc


def _prep_inputs(query, keys, Wa_w, Wa_b, Ua_w, Ua_b, Va_w, Va_b):
    """Host-side layout prep + per-core sharding."""
    keys_bf = np.ascontiguousarray(keys).astype(_BF16)              # [B, S, H]
    queryT_bf = np.ascontiguousarray(query.T).astype(_BF16)         # [H, B]
    uaT_bf = np.ascontiguousarray(Ua_w.T).astype(_BF16)             # [h, o]
    waT_bf = np.ascontiguousarray(Wa_w.T).astype(_BF16)             # [h, o]
    vacol_bf = np.ascontiguousarray(Va_w[0].reshape(HB, 128).T).astype(_BF16)
    biasc = np.ascontiguousarray(
        (Wa_b + Ua_b).astype(np.float32).reshape(HB, 128).T
    )  # [128, HB]

    in_maps = []
    for c in range(NCORES):
        sl = slice(c * BPC, (c + 1) * BPC)
        wq = np.ascontiguousarray(
            np.concatenate([waT_bf, queryT_bf[:, sl]], axis=1)
        )  # [H, H+BPC]
        in_maps.append(
            {
                "keysb": np.ascontiguousarray(keys_bf[sl]),
                "wq": wq,
                "uaT": uaT_bf,
                "vacol": vacol_bf,
                "biasc": biasc,
            }
        )
    return in_maps


def kernel(query, keys, Wa_w, Wa_b, Ua_w, Ua_b, Va_w, Va_b):
    global LAST_RESULTS
    from concourse import bass_utils

    if "nc" not in _CACHE:
        _CACHE["nc"] = _build()
    nc = _CACHE["nc"]

    in_maps = _prep_inputs(query, keys, Wa_w, Wa_b, Ua_w, Ua_b, Va_w, Va_b)
    res = bass_utils.run_bass_kernel_spmd(
        nc,
        in_maps,
        core_ids=list(range(NCORES)),
        trace=bool(os.environ.get("BASS_TRACE")),
    )
    LAST_RESULTS = res

    context = np.concatenate([r["out_ctx"] for r in res.results], axis=0)
    weights = np.concatenate([r["out_w"] for r in res.results], axis=0)
    return (
        context.reshape(B, 1, H).astype(np.float32),
        weights.reshape(B, 1, S).astype(np.float32),
    )


# revision 14
# speedup vs baseline: 10763.2399x; 10763.2399x over previous
"""Bahdanau attention Trainium2 kernel.

Problem: B=32, S=2048, H=1024 (fp32)
  q_proj = query @ Wa_w.T + Wa_b                  [B, H]
  k_proj = keys @ Ua_w.T + Ua_b                   [B, S, H]
  scores = tanh(q_proj + k_proj) @ Va_w[0] + Va_b [B, S]
  weights = softmax(scores, axis=1)               [B, 1, S]
  context = weights @ keys                        [B, 1, H]
returns (context, weights)

Sharding: data-parallel over batch, 4 examples per core on 8 cores.

Per-core device strategy (all matmuls in bf16, fp32 accumulation):
  - keys DMA'd twice per example: natural layout [s, h] (for context) and
    transposed [h, s] via the xbar DMA-transpose (for k_proj, which
    contracts over h so h must be on partitions).
  - k_proj computed per (o-block 128, s-chunk 512) into PSUM; ScalarE does
    tanh fused with the per-partition bias (q_proj[o] + Wa_b[o] + Ua_b[o])
    writing bf16 to SBUF.
  - scores via PE: Va as a [128,1] stationary column, contract o-blocks.
  - softmax on a single partition row [1, 2048] (Va_b omitted: softmax is
    shift-invariant so it cannot affect either output).
  - column-layout rearrangements (q_proj row -> per-partition bias columns,
    weights row -> [128, 16] stationary column) go through small DRAM
    round-trip DMAs: PE-transpose would put >1 sem wait on the S3_LW
    struct, which walrus rejects.
"""

import os
import sys

sys.path.insert(0, "/opt/trn_rl_repo")

import numpy as np
import ml_dtypes

B, S, H = 32, 2048, 1024
NCORES = 8
BPC = B // NCORES  # examples per core = 4
HB = H // 128      # h (and o) 128-blocks = 8
SBLK = S // 128    # s 128-blocks = 16
SC = 4             # s-chunks per example
SCW = S // SC      # s-chunk width = 512

_BF16 = ml_dtypes.bfloat16

_CACHE = {}
LAST_RESULTS = None  # test harness reads profile/exec time from here


def _build():
    import concourse.bacc as bacc
    import concourse.tile as tile
    from concourse import mybir

    f32 = mybir.dt.float32
    bf16 = mybir.dt.bfloat16
    AF = mybir.ActivationFunctionType
    AX = mybir.AxisListType

    nc = bacc.Bacc("TRN2", target_bir_lowering=False, debug=False)

    keysb = nc.dram_tensor("keysb", [BPC, S, H], bf16, kind="ExternalInput").ap()
    # waT [h, o] and queryT [h, b] packed along the free dim -> one DMA,
    # one semaphore for the q_proj matmul group.
    wq = nc.dram_tensor("wq", [H, H + BPC], bf16, kind="ExternalInput").ap()
    uaT = nc.dram_tensor("uaT", [H, H], bf16, kind="ExternalInput").ap()
    vacol = nc.dram_tensor("vacol", [128, HB], bf16, kind="ExternalInput").ap()
    biasc = nc.dram_tensor("biasc", [128, HB], f32, kind="ExternalInput").ap()
    out_ctx = nc.dram_tensor("out_ctx", [BPC, H], f32, kind="ExternalOutput").ap()
    out_w = nc.dram_tensor("out_w", [BPC, S], f32, kind="ExternalOutput").ap()
    # DRAM scratch for row->column rearrangement round trips
    qp_dram = nc.dram_tensor("qp_dram", [BPC, H], f32, kind="Internal").ap()
    w_dram = nc.dram_tensor("w_dram", [BPC, S], f32, kind="Internal").ap()

    with tile.TileContext(nc) as tc:
        with (
            tc.tile_pool(name="consts", bufs=1) as consts,
            tc.tile_pool(name="keys", bufs=2) as keys_pool,
            tc.tile_pool(name="th", bufs=2) as th_pool,
            tc.tile_pool(name="sm", bufs=2) as sm_pool,
            tc.tile_pool(name="pkp", bufs=2, space="PSUM") as pkp_pool,
            tc.tile_pool(name="psc", bufs=2, space="PSUM") as psc_pool,
            tc.tile_pool(name="pacc", bufs=2, space="PSUM") as pacc_pool,
        ):
            # ---- constants ----
            uaT_sb = consts.tile([128, HB, H], bf16)
            nc.sync.dma_start(out=uaT_sb, in_=uaT.rearrange("(i p) o -> p i o", p=128))
            wq_sb = consts.tile([128, HB, H + BPC], bf16)
            nc.sync.dma_start(out=wq_sb, in_=wq.rearrange("(i p) o -> p i o", p=128))
            vacol_sb = consts.tile([128, HB], bf16)
            nc.sync.dma_start(out=vacol_sb, in_=vacol)
            biasc_sb = consts.tile([128, HB], f32)
            nc.sync.dma_start(out=biasc_sb, in_=biasc)

            # ---- q_proj for all examples: psum_qp[b, o] = queryT.T @ waT ----
            psum_qp = pacc_pool.tile([BPC, H], f32, tag="acc")
            for n in range(2):
                for i in range(HB):
                    nc.tensor.matmul(
                        psum_qp[:, n * 512 : (n + 1) * 512],
                        lhsT=wq_sb[:, i, H : H + BPC],
                        rhs=wq_sb[:, i, n * 512 : (n + 1) * 512],
                        start=(i == 0),
                        stop=(i == HB - 1),
                    )
            qp_sb = sm_pool.tile([BPC, H], f32, tag="qp", bufs=1)
            nc.vector.tensor_copy(qp_sb, psum_qp)
            # row -> column layout via DRAM round trip, then add combined bias
            nc.sync.dma_start(out=qp_dram, in_=qp_sb)
            qpT_raw = consts.tile([128, BPC, HB], f32)
            with nc.allow_non_contiguous_dma("one-time 16KB gather"):
                nc.sync.dma_start(
                    out=qpT_raw, in_=qp_dram.rearrange("b (j p) -> p b j", p=128)
                )
            qpT_sb = consts.tile([128, BPC, HB], f32)
            for j in range(HB):
                nc.vector.tensor_scalar_add(
                    qpT_sb[:, :, j], qpT_raw[:, :, j], biasc_sb[:, j : j + 1]
                )

            # ---- per-example main loop (software-pipelined: example b's
            # context matmuls are emitted during example b+1 so the PE queue
            # never stalls on the softmax/weights-gather chain) ----
            def emit_ctx(knat, wcol, b):
                pctx = pacc_pool.tile([1, H], f32, tag="acc", name="pctx")
                for n in range(2):
                    for k in range(SBLK):
                        nc.tensor.matmul(
                            pctx[:, n * 512 : (n + 1) * 512],
                            lhsT=wcol[:, k : k + 1],
                            rhs=knat[:, k, n * 512 : (n + 1) * 512],
                            start=(k == 0),
                            stop=(k == SBLK - 1),
                        )
                ctx_sb = sm_pool.tile([1, H], f32, tag="ctx", bufs=2, name="ctx_sb")
                nc.vector.tensor_copy(ctx_sb, pctx)
                nc.sync.dma_start(out=out_ctx[b : b + 1], in_=ctx_sb)

            prev = None
            for b in range(BPC):
                knat = keys_pool.tile([128, SBLK, H], bf16, tag="knat", bufs=2)

                scores = sm_pool.tile([1, S], f32, tag="scores", bufs=2)
                for sc in range(SC):
                    # one tile per h-block so each matmul waits on exactly one
                    # transpose-DMA completion
                    kTs = []
                    for i in range(HB):
                        kTi = keys_pool.tile(
                            [128, SCW], bf16, tag=f"kT{i}", bufs=2, name=f"kT{i}"
                        )
                        nc.sync.dma_start_transpose(
                            out=kTi,
                            in_=keysb[
                                b, sc * SCW : (sc + 1) * SCW, i * 128 : (i + 1) * 128
                            ],
                        )
                        kTs.append(kTi)
                    if sc == 0:
                        # knat isn't needed until this example's (deferred)
                        # context matmuls; issue it after the first chunk's
                        # transpose DMAs so they own the critical path
                        nc.sync.dma_start(
                            out=knat,
                            in_=keysb[b].rearrange("(k p) h -> p k h", p=128),
                        )
                    th = th_pool.tile([128, HB, SCW], bf16, tag="th")
                    for j in range(HB):
                        pk = pkp_pool.tile([128, SCW], f32, tag="pk")
                        for i in range(HB):
                            nc.tensor.matmul(
                                pk,
                                lhsT=uaT_sb[:, i, j * 128 : (j + 1) * 128],
                                rhs=kTs[i],
                                start=(i == 0),
                                stop=(i == HB - 1),
                            )
                        nc.scalar.activation(
                            th[:, j, :], pk, AF.Tanh, bias=qpT_sb[:, b, j : j + 1]
                        )
                    ps = psc_pool.tile([1, SCW], f32, tag="ps")
                    for j in range(HB):
                        nc.tensor.matmul(
                            ps,
                            lhsT=vacol_sb[:, j : j + 1],
                            rhs=th[:, j, :],
                            start=(j == 0),
                            stop=(j == HB - 1),
                        )
                    nc.vector.tensor_copy(scores[:, sc * SCW : (sc + 1) * SCW], ps)

                # softmax on [1, S] (single partition)
                nmax = sm_pool.tile([1, 1], f32, tag="nmax")
                nc.vector.reduce_max(nmax, scores, axis=AX.X, negate=True)
                e = sm_pool.tile([1, S], f32, tag="e", bufs=2)
                esum = sm_pool.tile([1, 1], f32, tag="esum")
                nc.scalar.activation(e, scores, AF.Exp, bias=nmax, accum_out=esum)
                rsum = sm_pool.tile([1, 1], f32, tag="rsum")
                nc.vector.reciprocal(rsum, esum)
                wts = sm_pool.tile([1, S], f32, tag="wts", bufs=2)
                nc.vector.tensor_scalar_mul(wts, e, rsum)
                nc.sync.dma_start(out=out_w[b : b + 1], in_=wts)

                # weights row -> bf16 column tile [128, SBLK] via DRAM round trip
                nc.sync.dma_start(out=w_dram[b : b + 1], in_=wts)
                wcol_f = sm_pool.tile([128, SBLK], f32, tag="wcol_f", bufs=2)
                with nc.allow_non_contiguous_dma("8KB gather per example"):
                    nc.sync.dma_start(
                        out=wcol_f, in_=w_dram[b].rearrange("(k p) -> p k", p=128)
                    )
                wcol = sm_pool.tile([128, SBLK], bf16, tag="wcol", bufs=2)
                nc.vector.tensor_copy(wcol, wcol_f)

                if prev is not None:
                    emit_ctx(*prev)
                prev = (knat, wcol, b)

            emit_ctx(*prev)

    nc.compile()
    return nc


def _prep_inputs(query, keys, Wa_w, Wa_b, Ua_w, Ua_b, Va_w, Va_b):
    """Host-side layout prep + per-core sharding."""
    keys_bf = np.ascontiguousarray(keys).astype(_BF16)              # [B, S, H]
    queryT_bf = np.ascontiguousarray(query.T).astype(_BF16)         # [H, B]
    uaT_bf = np.ascontiguousarray(Ua_w.T).astype(_BF16)             # [h, o]
    waT_bf = np.ascontiguousarray(Wa_w.T).astype(_BF16)             # [h, o]
    vacol_bf = np.ascontiguousarray(Va_w[0].reshape(HB, 128).T).astype(_BF16)
    biasc = np.ascontiguousarray(
        (Wa_b + Ua_b).astype(np.float32).reshape(HB, 128).T
    )  # [128, HB]

    in_maps = []
    for c in range(NCORES):
        sl = slice(c * BPC, (c + 1) * BPC)
        wq = np.ascontiguousarray(
            np.concatenate([waT_bf, queryT_bf[:, sl]], axis=1)
        )  # [H, H+BPC]
        in_maps.append(
            {
                "keysb": np.ascontiguousarray(keys_bf[sl]),
                "wq": wq,
                "uaT": uaT_bf,
                "vacol": vacol_bf,
                "biasc": biasc,
            }
        )
    return in_maps


def kernel(query, keys, Wa_w, Wa_b, Ua_w, Ua_b, Va_w, Va_b):
    global LAST_RESULTS
    from concourse import bass_utils

    if "nc" not in _CACHE:
        _CACHE["nc"] = _build()
    nc = _CACHE["nc"]

    in_maps = _prep_inputs(query, keys, Wa_w, Wa_b, Ua_w, Ua_b, Va_w, Va_b)
    res = bass_utils.run_bass_kernel_spmd(
        nc,
        in_maps,
        core_ids=list(range(NCORES)),
        trace=bool(os.environ.get("BASS_TRACE")),
    )
    LAST_RESULTS = res

    context = np.concatenate([r["out_ctx"] for r in res.results], axis=0)
    weights = np.concatenate([r["out_w"] for r in res.results], axis=0)
    return (
        context.reshape(B, 1, H).astype(np.float32),
        weights.reshape(B, 1, S).astype(np.float32),
    )
